# revision 3
# baseline (speedup 1.0000x reference)
"""ChildSum TreeLSTM (complete binary tree, depth 17) on 8 Trainium2 NeuronCores.

Strategy
--------
* The tree below level 3 consists of 8 independent subtrees (roots = nodes
  7..14).  Core m processes the full subtree of node 7+m, bottom-up from the
  leaves (level 16) to level L_STOP.  Zero cross-core communication.
* Everything on-device lives in a feature-major ("transposed") layout:
  [128 hidden units on partitions, nodes on the free axis].  The host
  pre-transposes x (and the weights) when building the per-core inputs, so
  the device never transposes anything.
* Within each level the nodes are stored in an "even/odd split" order:
  child-0 of all stored parents first, then child-1 of all stored parents.
  This makes every device-side slice contiguous: the child-sum becomes two
  accumulating matmuls, and the per-child forget gates line up with the two
  contiguous halves of the child buffer.  The permutation is applied by the
  host while slicing x - free on device.
* Gate GEMMs run as float32r (full-rate PE) accumulating into fp32 PSUM:
      i|o|u: psum = W@x + U@h_even + U@h_odd   (child sum folded into PE)
      f0|f1: psum = W@x + U@h_child
  Sigmoid/Tanh run on the scalar engine with the (combined) biases fused in,
  products and adds on the vector engine, all fp32.
* The top of the tree (levels L_STOP-1 .. 0, 2**L_STOP - 1 nodes out of
  131071) is finished on the host in float64 - a negligible tail that would
  otherwise serialize the device on tiny tensors.
"""

import os
import sys

import numpy as np

for _p in ("/opt/trn_rl_repo", "/root/.axon_site/_ro/trn_rl_repo"):
    if os.path.isdir(_p) and _p not in sys.path:
        sys.path.insert(0, _p)

import concourse.bacc as bacc
import concourse.tile as tile
from concourse import mybir
from concourse.bass_utils import run_bass_kernel_spmd

DEPTH = 17
N = 2**DEPTH - 1
H = 128
NCORES = 8
L_STOP = int(os.environ.get("KERNEL_L_STOP", "8"))  # lowest level computed on device
CHUNK = 512

DEV_LEVELS = list(range(DEPTH - 1, L_STOP - 1, -1))  # 16 .. L_STOP
LCOLS = {d: (2**d) // NCORES for d in DEV_LEVELS}  # per-core cols per level
XCOLS = sum(LCOLS.values())
XOFF = {}
_off = 0
for _d in DEV_LEVELS:
    XOFF[_d] = _off
    _off += LCOLS[_d]
TOPC = LCOLS[L_STOP]

F32 = mybir.dt.float32
F32R = mybir.dt.float32r

W_NAMES = ["Wi", "Wo", "Wu", "Wf", "Ui", "Uo", "Uu", "Uf"]
WOFF = {n: i * H for i, n in enumerate(W_NAMES)}


def _build_nc():
    nc = bacc.Bacc("TRN2", target_bir_lowering=False, debug=False)
    xT = nc.dram_tensor("xT", [H, XCOLS], F32R, kind="ExternalInput").ap()
    wT = nc.dram_tensor("wT", [H, 8 * H], F32R, kind="ExternalInput").ap()
    bias = nc.dram_tensor("bias", [H, 8], F32, kind="ExternalInput").ap()
    hc = nc.dram_tensor("hc", [H, 2 * TOPC], F32, kind="ExternalOutput").ap()

    Sig = mybir.ActivationFunctionType.Sigmoid
    Tanh = mybir.ActivationFunctionType.Tanh

    with tile.TileContext(nc) as tc:
        with (
            tc.tile_pool(name="const", bufs=1) as constp,
            tc.tile_pool(name="hcbuf", bufs=1) as hcp,
            tc.tile_pool(name="xin", bufs=3) as xinp,
            tc.tile_pool(name="gates", bufs=2) as gp,
            tc.tile_pool(name="ps2", bufs=2, space="PSUM") as ps2,
            tc.tile_pool(name="ps1", bufs=1, space="PSUM") as ps1,
        ):
            w_sb = constp.tile([H, 8 * H], F32R, tag="w")
            nc.sync.dma_start(out=w_sb, in_=wT)
            b_sb = constp.tile([H, 8], F32, tag="b")
            nc.sync.dma_start(out=b_sb, in_=bias)
            # bias cols: 0:bi_leaf 1:bo_leaf 2:bu_leaf 3:bi 4:bo 5:bu 6:bf

            hA = hcp.tile([H, LCOLS[DEPTH - 1]], F32R, tag="hA")
            cA = hcp.tile([H, LCOLS[DEPTH - 1]], F32, tag="cA")
            hB = hcp.tile([H, LCOLS[DEPTH - 2]], F32R, tag="hB")
            cB = hcp.tile([H, LCOLS[DEPTH - 2]], F32, tag="cB")

            def wsl(name):
                return w_sb[:, WOFF[name] : WOFF[name] + H]

            mm = nc.tensor.matmul
            act = nc.scalar.activation
            tt = nc.vector

            for d in DEV_LEVELS:
                L = LCOLS[d]
                leaf = d == DEPTH - 1
                even_lvl = (DEPTH - 1 - d) % 2 == 0
                h_out, c_out = (hA, cA) if even_lvl else (hB, cB)
                h_in, c_in = (hB, cB) if even_lvl else (hA, cA)
                for a in range(0, L, CHUNK):
                    C = min(CHUNK, L - a)
                    x_t = xinp.tile([H, CHUNK], F32R, tag="x")
                    nc.sync.dma_start(
                        out=x_t[:, :C], in_=xT[:, XOFF[d] + a : XOFF[d] + a + C]
                    )
                    io_ps = ps2.tile([H, 2 * CHUNK], F32, tag="io")
                    u_ps = ps2.tile([H, CHUNK], F32, tag="u")
                    if leaf:
                        mm(io_ps[:, :C], wsl("Wi"), x_t[:, :C], start=True, stop=True)
                        mm(
                            io_ps[:, CHUNK : CHUNK + C],
                            wsl("Wo"),
                            x_t[:, :C],
                            start=True,
                            stop=True,
                        )
                        mm(u_ps[:, :C], wsl("Wu"), x_t[:, :C], start=True, stop=True)
                    else:
                        he = h_in[:, a : a + C]
                        ho = h_in[:, L + a : L + a + C]
                        isl = io_ps[:, :C]
                        osl = io_ps[:, CHUNK : CHUNK + C]
                        mm(isl, wsl("Wi"), x_t[:, :C], start=True, stop=False)
                        mm(isl, wsl("Ui"), he, start=False, stop=False)
                        mm(isl, wsl("Ui"), ho, start=False, stop=True)
                        mm(osl, wsl("Wo"), x_t[:, :C], start=True, stop=False)
                        mm(osl, wsl("Uo"), he, start=False, stop=False)
                        mm(osl, wsl("Uo"), ho, start=False, stop=True)
                        mm(u_ps[:, :C], wsl("Wu"), x_t[:, :C], start=True, stop=False)
                        mm(u_ps[:, :C], wsl("Uu"), he, start=False, stop=False)
                        mm(u_ps[:, :C], wsl("Uu"), ho, start=False, stop=True)
                        f_ps = ps1.tile([H, 2 * CHUNK], F32, tag="f")
                        f0 = f_ps[:, :C]
                        f1 = f_ps[:, CHUNK : CHUNK + C]
                        mm(f0, wsl("Wf"), x_t[:, :C], start=True, stop=False)
                        mm(f1, wsl("Wf"), x_t[:, :C], start=True, stop=False)
                        mm(f0, wsl("Uf"), he, start=False, stop=True)
                        mm(f1, wsl("Uf"), ho, start=False, stop=True)

                    io_sb = gp.tile([H, 2 * CHUNK], F32, tag="io_sb")
                    u_sb = gp.tile([H, CHUNK], F32, tag="u_sb")
                    bcol = 0 if leaf else 3
                    act(io_sb[:, :C], io_ps[:, :C], Sig, bias=b_sb[:, bcol : bcol + 1])
                    act(
                        io_sb[:, CHUNK : CHUNK + C],
                        io_ps[:, CHUNK : CHUNK + C],
                        Sig,
                        bias=b_sb[:, bcol + 1 : bcol + 2],
                    )
                    act(
                        u_sb[:, :C],
                        u_ps[:, :C],
                        Tanh,
                        bias=b_sb[:, bcol + 2 : bcol + 3],
                    )
                    c_sl = c_out[:, a : a + C]
                    if leaf:
                        tt.tensor_mul(c_sl, io_sb[:, :C], u_sb[:, :C])
                    else:
                        f_sb = gp.tile([H, 2 * CHUNK], F32, tag="f_sb")
                        if C == CHUNK:
                            act(f_sb, f_ps, Sig, bias=b_sb[:, 6:7])
                        else:
                            act(f_sb[:, :C], f_ps[:, :C], Sig, bias=b_sb[:, 6:7])
                            act(
                                f_sb[:, CHUNK : CHUNK + C],
                                f_ps[:, CHUNK : CHUNK + C],
                                Sig,
                                bias=b_sb[:, 6:7],
                            )
                        q = gp.tile([H, CHUNK], F32, tag="q")
                        pr = gp.tile([H, 2 * CHUNK], F32, tag="pr")
                        s1 = gp.tile([H, CHUNK], F32, tag="s1")
                        tt.tensor_mul(q[:, :C], io_sb[:, :C], u_sb[:, :C])
                        tt.tensor_mul(pr[:, :C], f_sb[:, :C], c_in[:, a : a + C])
                        tt.tensor_mul(
                            pr[:, CHUNK : CHUNK + C],
                            f_sb[:, CHUNK : CHUNK + C],
                            c_in[:, L + a : L + a + C],
                        )
                        tt.tensor_add(s1[:, :C], q[:, :C], pr[:, :C])
                        tt.tensor_add(c_sl, s1[:, :C], pr[:, CHUNK : CHUNK + C])
                    t_sb = gp.tile([H, CHUNK], F32, tag="t_sb")
                    act(t_sb[:, :C], c_sl, Tanh)
                    tt.tensor_mul(
                        h_out[:, a : a + C], io_sb[:, CHUNK : CHUNK + C], t_sb[:, :C]
                    )

            nc.sync.dma_start(out=hc[:, :TOPC], in_=hA[:, :TOPC].bitcast(F32))
            nc.sync.dma_start(out=hc[:, TOPC : 2 * TOPC], in_=cA[:, :TOPC])
    nc.finalize()
    return nc


_NC = None


def _get_nc():
    global _NC
    if _NC is None:
        _NC = _build_nc()
    return _NC


def _stored_cols(m):
    """Column order (node ids) of core m's xT buffer: levels 16..L_STOP,
    each level in even/odd-split order derived from the level above."""
    ids = np.arange(2**L_STOP - 1 + TOPC * m, 2**L_STOP - 1 + TOPC * (m + 1))
    per_level = {L_STOP: ids}
    for d in range(L_STOP, DEPTH - 1):
        ids = np.concatenate([2 * ids + 1, 2 * ids + 2])
        per_level[d + 1] = ids
    return np.concatenate([per_level[d] for d in DEV_LEVELS]), per_level


def _sigmoid(z):
    return 1.0 / (1.0 + np.exp(-z))


def kernel(**inputs):
    x = np.ascontiguousarray(np.asarray(inputs["x"], dtype=np.float32))
    wstack = np.ascontiguousarray(
        np.concatenate([np.asarray(inputs[n], np.float32).T for n in W_NAMES], axis=1)
    )
    b = {k: np.asarray(inputs[k], np.float64) for k in inputs if k.startswith("b")}
    bias = np.zeros((H, 8), np.float32)
    bias[:, 0] = b["bWi"]
    bias[:, 1] = b["bWo"]
    bias[:, 2] = b["bWu"]
    bias[:, 3] = b["bWi"] + b["bUi"]
    bias[:, 4] = b["bWo"] + b["bUo"]
    bias[:, 5] = b["bWu"] + b["bUu"]
    bias[:, 6] = b["bWf"] + b["bUf"]

    in_maps = []
    for m in range(NCORES):
        cols, _ = _stored_cols(m)
        in_maps.append(
            {
                "xT": np.ascontiguousarray(x[cols].T),
                "wT": wstack,
                "bias": bias,
            }
        )

    nc = _get_nc()
    trace = bool(int(os.environ.get("KERNEL_TRACE", "0")))
    try:
        res = run_bass_kernel_spmd(
            nc, in_maps, core_ids=list(range(NCORES)), trace=trace
        )
    except ModuleNotFoundError:
        res = run_bass_kernel_spmd(nc, in_maps, core_ids=list(range(NCORES)))
    if trace and res.exec_time_ns is not None:
        print(f"HW exec time: {res.exec_time_ns} ns")

    # stored level-L_STOP columns of core m are the natural-order nodes
    # 2**L_STOP - 1 + 32*m ...  (that's how _stored_cols seeds them)
    h_next = np.concatenate(
        [res.results[m]["hc"][:, :TOPC] for m in range(NCORES)], axis=1
    ).T.astype(np.float64)
    c_next = np.concatenate(
        [res.results[m]["hc"][:, TOPC : 2 * TOPC] for m in range(NCORES)], axis=1
    ).T.astype(np.float64)

    # finish levels L_STOP-1 .. 0 on the host (float64)
    xd = x.astype(np.float64)
    W = {n: np.asarray(inputs[n], np.float64) for n in W_NAMES}
    for d in range(L_STOP - 1, -1, -1):
        s = 2**d - 1
        cnt = 2**d
        xs = xd[s : s + cnt]
        li = xs @ W["Wi"].T + b["bWi"]
        lf = xs @ W["Wf"].T + b["bWf"]
        lo = xs @ W["Wo"].T + b["bWo"]
        lu = xs @ W["Wu"].T + b["bWu"]
        ch_h = h_next.reshape(cnt, 2, H)
        ch_c = c_next.reshape(cnt, 2, H)
        hs = ch_h[:, 0, :] + ch_h[:, 1, :]
        i = _sigmoid(li + hs @ W["Ui"].T + b["bUi"])
        o = _sigmoid(lo + hs @ W["Uo"].T + b["bUo"])
        u = np.tanh(lu + hs @ W["Uu"].T + b["bUu"])
        f0 = _sigmoid(lf + ch_h[:, 0, :] @ W["Uf"].T + b["bUf"])
        f1 = _sigmoid(lf + ch_h[:, 1, :] @ W["Uf"].T + b["bUf"])
        c = i * u + f0 * ch_c[:, 0, :] + f1 * ch_c[:, 1, :]
        h = o * np.tanh(c)
        h_next, c_next = h, c

    out = h_next[0] @ np.asarray(inputs["Wp"], np.float64).T + np.asarray(
        inputs["bWp"], np.float64
    )
    return out.astype(np.float32)


# revision 20
# speedup vs baseline: 1.1889x; 1.1889x over previous
"""ChildSum TreeLSTM (complete binary tree, depth 17) on 8 Trainium2 NeuronCores.

Strategy
--------
* The tree below level 3 consists of 8 independent subtrees (roots = nodes
  7..14).  Core m processes the full subtree of node 7+m, bottom-up from the
  leaves (level 16) to level L_STOP.  Zero cross-core communication.
* Everything on-device lives in a feature-major ("transposed") layout:
  [128 hidden units on partitions, nodes on the free axis].  The host
  pre-transposes x (and the weights) when building the per-core inputs, so
  the device never transposes anything.
* Within each level the nodes are stored in an "even/odd split" order:
  child-0 of all stored parents first, then child-1 of all stored parents.
  This makes every device-side slice contiguous: the child-sum becomes two
  accumulating matmuls, and the per-child forget gates line up with the two
  contiguous halves of the child buffer.  The permutation is applied by the
  host while slicing x - free on device.
* Gate GEMMs run as float32r (full-rate PE) accumulating into fp32 PSUM:
      i|o|u: psum = W@x + U@h_even + U@h_odd   (child sum folded into PE)
      f0|f1: psum = W@x + U@h_child
  Sigmoid/Tanh run on the scalar engine with the (combined) biases fused in,
  products and adds on the vector engine, all fp32.
* The top of the tree (levels L_STOP-1 .. 0, 2**L_STOP - 1 nodes out of
  131071) is finished on the host in float64 - a negligible tail that would
  otherwise serialize the device on tiny tensors.
"""

import os
import sys

import numpy as np

for _p in ("/opt/trn_rl_repo", "/root/.axon_site/_ro/trn_rl_repo"):
    if os.path.isdir(_p) and _p not in sys.path:
        sys.path.insert(0, _p)

import concourse.bacc as bacc
import concourse.tile as tile
from concourse import mybir
from concourse.bass_utils import run_bass_kernel_spmd

DEPTH = 17
N = 2**DEPTH - 1
H = 128
NCORES = 8
L_STOP = int(os.environ.get("KERNEL_L_STOP", "12"))  # lowest level computed on device
GPSIMD_LEAF = bool(int(os.environ.get("KERNEL_GPSIMD_LEAF", "1")))
CHUNK = 512

DEV_LEVELS = list(range(DEPTH - 1, L_STOP - 1, -1))  # 16 .. L_STOP
LCOLS = {d: (2**d) // NCORES for d in DEV_LEVELS}  # per-core cols per level
XCOLS = sum(LCOLS.values())
XOFF = {}
_off = 0
for _d in DEV_LEVELS:
    XOFF[_d] = _off
    _off += LCOLS[_d]
TOPC = LCOLS[L_STOP]

F32 = mybir.dt.float32
F32R = mybir.dt.float32r

W_NAMES = ["Wi", "Wo", "Wu", "Wf", "Ui", "Uo", "Uu", "Uf"]
WOFF = {n: i * H for i, n in enumerate(W_NAMES)}


def _build_nc():
    nc = bacc.Bacc("TRN2", target_bir_lowering=False, debug=False)
    xT = nc.dram_tensor("xT", [H, XCOLS], F32R, kind="ExternalInput").ap()
    wT = nc.dram_tensor("wT", [H, 8 * H], F32R, kind="ExternalInput").ap()
    bias = nc.dram_tensor("bias", [H, 8], F32, kind="ExternalInput").ap()
    hc = nc.dram_tensor("hc", [H, 2 * TOPC], F32, kind="ExternalOutput").ap()

    Sig = mybir.ActivationFunctionType.Sigmoid
    Tanh = mybir.ActivationFunctionType.Tanh

    with tile.TileContext(nc) as tc:
        with (
            tc.tile_pool(name="const", bufs=1) as constp,
            tc.tile_pool(name="hcbuf", bufs=1) as hcp,
            tc.tile_pool(name="xin", bufs=4) as xinp,
            tc.tile_pool(name="gates", bufs=2) as gp,
            tc.tile_pool(name="ps2", bufs=2, space="PSUM") as ps2,
            tc.tile_pool(name="ps1", bufs=1, space="PSUM") as ps1,
        ):
            # weights/bias go through the gpsimd (SWDGE) queue so the x-chunk
            # stream on the sync queue starts immediately
            w_sb = constp.tile([H, 8 * H], F32R, tag="w")
            nc.sync.dma_start(out=w_sb[:, : 3 * H], in_=wT[:, : 3 * H])
            nc.gpsimd.dma_start(out=w_sb[:, 3 * H :], in_=wT[:, 3 * H :])
            b_sb = constp.tile([H, 8], F32, tag="b")
            nc.gpsimd.dma_start(out=b_sb, in_=bias)
            # warm the sigmoid/tanh ACT table at t=0 so the ~1.3us table load
            # is off the critical path of the first real activation
            warm = constp.tile([H, 1], F32, tag="warm")
            nc.vector.memset(warm, 0.0)
            nc.scalar.activation(
                warm, warm, mybir.ActivationFunctionType.Sigmoid
            )
            # bias cols: 0:bi_leaf 1:bo_leaf 2:bu_leaf 3:bi 4:bo 5:bu 6:bf

            # dedicated per-level h/c buffers: each written once, read once -
            # no write-after-read hazards serializing level boundaries
            hbuf = {
                d: hcp.tile([H, LCOLS[d]], F32R, tag=f"h{d}", name=f"h{d}")
                for d in DEV_LEVELS
            }
            cbuf = {
                d: hcp.tile([H, LCOLS[d]], F32, tag=f"c{d}", name=f"c{d}")
                for d in DEV_LEVELS
            }

            def wsl(name):
                return w_sb[:, WOFF[name] : WOFF[name] + H]

            mm = nc.tensor.matmul
            act = nc.scalar.activation
            tt = nc.vector

            # tanh(c)+h of a chunk are deferred by one chunk (software
            # pipelining) so the scalar engine never stalls on the vector
            # engine's c-chain for the chunk it just fed
            pending = []

            def flush_pending():
                while pending:
                    c_sl, o_ap, h_sl, Cp = pending.pop()
                    t_sb = gp.tile([H, CHUNK], F32, tag="t_sb", name="t_sb")
                    act(t_sb[:, :Cp], c_sl, Tanh)
                    tt.tensor_mul(h_sl, o_ap, t_sb[:, :Cp])

            for d in DEV_LEVELS:
                L = LCOLS[d]
                leaf = d == DEPTH - 1
                h_out, c_out = hbuf[d], cbuf[d]
                h_in, c_in = (None, None) if leaf else (hbuf[d + 1], cbuf[d + 1])
                # the child level's last deferred tanh/h must be emitted before
                # any matmul of this level reads it (deps follow emission order)
                flush_pending()
                step = 256 if L <= 1024 else CHUNK
                for a in range(0, L, step):
                    C = min(step, L - a)
                    x_t = xinp.tile([H, CHUNK], F32R, tag="x")
                    nc.sync.dma_start(
                        out=x_t[:, :C], in_=xT[:, XOFF[d] + a : XOFF[d] + a + C]
                    )
                    io_ps = ps2.tile([H, 2 * CHUNK], F32, tag="io")
                    u_ps = ps2.tile([H, CHUNK], F32, tag="u")
                    if leaf:
                        mm(io_ps[:, :C], wsl("Wi"), x_t[:, :C], start=True, stop=True)
                        mm(
                            io_ps[:, CHUNK : CHUNK + C],
                            wsl("Wo"),
                            x_t[:, :C],
                            start=True,
                            stop=True,
                        )
                        mm(u_ps[:, :C], wsl("Wu"), x_t[:, :C], start=True, stop=True)
                    else:
                        he = h_in[:, a : a + C]
                        ho = h_in[:, L + a : L + a + C]
                        isl = io_ps[:, :C]
                        osl = io_ps[:, CHUNK : CHUNK + C]
                        mm(isl, wsl("Wi"), x_t[:, :C], start=True, stop=False)
                        mm(isl, wsl("Ui"), he, start=False, stop=False)
                        mm(isl, wsl("Ui"), ho, start=False, stop=True)
                        mm(osl, wsl("Wo"), x_t[:, :C], start=True, stop=False)
                        mm(osl, wsl("Uo"), he, start=False, stop=False)
                        mm(osl, wsl("Uo"), ho, start=False, stop=True)
                        mm(u_ps[:, :C], wsl("Wu"), x_t[:, :C], start=True, stop=False)
                        mm(u_ps[:, :C], wsl("Uu"), he, start=False, stop=False)
                        mm(u_ps[:, :C], wsl("Uu"), ho, start=False, stop=True)
                        f_ps = ps1.tile([H, 2 * CHUNK], F32, tag="f")
                        f0 = f_ps[:, :C]
                        f1 = f_ps[:, CHUNK : CHUNK + C]
                        mm(f0, wsl("Wf"), x_t[:, :C], start=True, stop=False)
                        mm(f1, wsl("Wf"), x_t[:, :C], start=True, stop=False)
                        mm(f0, wsl("Uf"), he, start=False, stop=True)
                        mm(f1, wsl("Uf"), ho, start=False, stop=True)

                    io_sb = gp.tile([H, 2 * CHUNK], F32, tag="io_sb")
                    u_sb = gp.tile([H, CHUNK], F32, tag="u_sb")
                    bcol = 0 if leaf else 3
                    act(io_sb[:, :C], io_ps[:, :C], Sig, bias=b_sb[:, bcol : bcol + 1])
                    act(
                        io_sb[:, CHUNK : CHUNK + C],
                        io_ps[:, CHUNK : CHUNK + C],
                        Sig,
                        bias=b_sb[:, bcol + 1 : bcol + 2],
                    )
                    act(
                        u_sb[:, :C],
                        u_ps[:, :C],
                        Tanh,
                        bias=b_sb[:, bcol + 2 : bcol + 3],
                    )
                    c_sl = c_out[:, a : a + C]
                    if leaf:
                        if GPSIMD_LEAF:
                            nc.gpsimd.tensor_mul(c_sl, io_sb[:, :C], u_sb[:, :C])
                        else:
                            tt.tensor_mul(c_sl, io_sb[:, :C], u_sb[:, :C])
                        flush_pending()
                    else:
                        f_sb = gp.tile([H, 2 * CHUNK], F32, tag="f_sb")
                        if C == CHUNK:
                            act(f_sb, f_ps, Sig, bias=b_sb[:, 6:7])
                        else:
                            act(f_sb[:, :C], f_ps[:, :C], Sig, bias=b_sb[:, 6:7])
                            act(
                                f_sb[:, CHUNK : CHUNK + C],
                                f_ps[:, CHUNK : CHUNK + C],
                                Sig,
                                bias=b_sb[:, 6:7],
                            )
                        q = gp.tile([H, CHUNK], F32, tag="q")
                        pr = gp.tile([H, 2 * CHUNK], F32, tag="pr")
                        s1 = gp.tile([H, CHUNK], F32, tag="s1")
                        nc.gpsimd.tensor_mul(q[:, :C], io_sb[:, :C], u_sb[:, :C])
                        tt.tensor_mul(pr[:, :C], f_sb[:, :C], c_in[:, a : a + C])
                        tt.tensor_mul(
                            pr[:, CHUNK : CHUNK + C],
                            f_sb[:, CHUNK : CHUNK + C],
                            c_in[:, L + a : L + a + C],
                        )
                        tt.tensor_add(s1[:, :C], q[:, :C], pr[:, :C])
                        tt.tensor_add(c_sl, s1[:, :C], pr[:, CHUNK : CHUNK + C])
                        flush_pending()
                    pending.append(
                        (c_sl, io_sb[:, CHUNK : CHUNK + C], h_out[:, a : a + C], C)
                    )

            flush_pending()

            nc.sync.dma_start(out=hc[:, :TOPC], in_=hbuf[L_STOP].bitcast(F32))
            nc.sync.dma_start(out=hc[:, TOPC : 2 * TOPC], in_=cbuf[L_STOP])
    nc.finalize()
    return nc


_NC = None


def _get_nc():
    global _NC
    if _NC is None:
        _NC = _build_nc()
    return _NC


def _stored_cols(m):
    """Column order (node ids) of core m's xT buffer: levels 16..L_STOP,
    each level in even/odd-split order derived from the level above."""
    ids = np.arange(2**L_STOP - 1 + TOPC * m, 2**L_STOP - 1 + TOPC * (m + 1))
    per_level = {L_STOP: ids}
    for d in range(L_STOP, DEPTH - 1):
        ids = np.concatenate([2 * ids + 1, 2 * ids + 2])
        per_level[d + 1] = ids
    return np.concatenate([per_level[d] for d in DEV_LEVELS]), per_level


def _sigmoid(z):
    return 1.0 / (1.0 + np.exp(-z))


def kernel(**inputs):
    x = np.ascontiguousarray(np.asarray(inputs["x"], dtype=np.float32))
    wstack = np.ascontiguousarray(
        np.concatenate([np.asarray(inputs[n], np.float32).T for n in W_NAMES], axis=1)
    )
    b = {k: np.asarray(inputs[k], np.float64) for k in inputs if k.startswith("b")}
    bias = np.zeros((H, 8), np.float32)
    bias[:, 0] = b["bWi"]
    bias[:, 1] = b["bWo"]
    bias[:, 2] = b["bWu"]
    bias[:, 3] = b["bWi"] + b["bUi"]
    bias[:, 4] = b["bWo"] + b["bUo"]
    bias[:, 5] = b["bWu"] + b["bUu"]
    bias[:, 6] = b["bWf"] + b["bUf"]

    in_maps = []
    for m in range(NCORES):
        cols, _ = _stored_cols(m)
        in_maps.append(
            {
                "xT": np.ascontiguousarray(x[cols].T),
                "wT": wstack,
                "bias": bias,
            }
        )

    nc = _get_nc()
    trace = bool(int(os.environ.get("KERNEL_TRACE", "0")))
    try:
        res = run_bass_kernel_spmd(
            nc, in_maps, core_ids=list(range(NCORES)), trace=trace
        )
    except ModuleNotFoundError:
        res = run_bass_kernel_spmd(nc, in_maps, core_ids=list(range(NCORES)))
    if trace and res.exec_time_ns is not None:
        print(f"HW exec time: {res.exec_time_ns} ns")

    # stored level-L_STOP columns of core m are the natural-order nodes
    # 2**L_STOP - 1 + 32*m ...  (that's how _stored_cols seeds them)
    h_next = np.concatenate(
        [res.results[m]["hc"][:, :TOPC] for m in range(NCORES)], axis=1
    ).T.astype(np.float64)
    c_next = np.concatenate(
        [res.results[m]["hc"][:, TOPC : 2 * TOPC] for m in range(NCORES)], axis=1
    ).T.astype(np.float64)

    # finish levels L_STOP-1 .. 0 on the host (float64)
    xd = x.astype(np.float64)
    W = {n: np.asarray(inputs[n], np.float64) for n in W_NAMES}
    for d in range(L_STOP - 1, -1, -1):
        s = 2**d - 1
        cnt = 2**d
        xs = xd[s : s + cnt]
        li = xs @ W["Wi"].T + b["bWi"]
        lf = xs @ W["Wf"].T + b["bWf"]
        lo = xs @ W["Wo"].T + b["bWo"]
        lu = xs @ W["Wu"].T + b["bWu"]
        ch_h = h_next.reshape(cnt, 2, H)
        ch_c = c_next.reshape(cnt, 2, H)
        hs = ch_h[:, 0, :] + ch_h[:, 1, :]
        i = _sigmoid(li + hs @ W["Ui"].T + b["bUi"])
        o = _sigmoid(lo + hs @ W["Uo"].T + b["bUo"])
        u = np.tanh(lu + hs @ W["Uu"].T + b["bUu"])
        f0 = _sigmoid(lf + ch_h[:, 0, :] @ W["Uf"].T + b["bUf"])
        f1 = _sigmoid(lf + ch_h[:, 1, :] @ W["Uf"].T + b["bUf"])
        c = i * u + f0 * ch_c[:, 0, :] + f1 * ch_c[:, 1, :]
        h = o * np.tanh(c)
        h_next, c_next = h, c

    out = h_next[0] @ np.asarray(inputs["Wp"], np.float64).T + np.asarray(
        inputs["bWp"], np.float64
    )
    return out.astype(np.float32)


# revision 22
# speedup vs baseline: 1.2016x; 1.0106x over previous
"""ChildSum TreeLSTM (complete binary tree, depth 17) on 8 Trainium2 NeuronCores.

Strategy
--------
* The tree below level 3 consists of 8 independent subtrees (roots = nodes
  7..14).  Core m processes the full subtree of node 7+m, bottom-up from the
  leaves (level 16) to level L_STOP.  Zero cross-core communication.
* Everything on-device lives in a feature-major ("transposed") layout:
  [128 hidden units on partitions, nodes on the free axis].  The host
  pre-transposes x (and the weights) when building the per-core inputs, so
  the device never transposes anything.
* Within each level the nodes are stored in an "even/odd split" order:
  child-0 of all stored parents first, then child-1 of all stored parents.
  This makes every device-side slice contiguous: the child-sum becomes two
  accumulating matmuls, and the per-child forget gates line up with the two
  contiguous halves of the child buffer.  The permutation is applied by the
  host while slicing x - free on device.
* Gate GEMMs run as float32r (full-rate PE) accumulating into fp32 PSUM:
      i|o|u: psum = W@x + U@h_even + U@h_odd   (child sum folded into PE)
      f0|f1: psum = W@x + U@h_child
  Sigmoid/Tanh run on the scalar engine with the (combined) biases fused in,
  products and adds on the vector engine, all fp32.
* The top of the tree (levels L_STOP-1 .. 0, 2**L_STOP - 1 nodes out of
  131071) is finished on the host in float64 - a negligible tail that would
  otherwise serialize the device on tiny tensors.
"""

import os
import sys

import numpy as np

for _p in ("/opt/trn_rl_repo", "/root/.axon_site/_ro/trn_rl_repo"):
    if os.path.isdir(_p) and _p not in sys.path:
        sys.path.insert(0, _p)

import concourse.bacc as bacc
import concourse.tile as tile
from concourse import mybir
from concourse.bass_utils import run_bass_kernel_spmd

DEPTH = 17
N = 2**DEPTH - 1
H = 128
NCORES = 8
L_STOP = int(os.environ.get("KERNEL_L_STOP", "12"))  # lowest level computed on device
GPSIMD_LEAF = bool(int(os.environ.get("KERNEL_GPSIMD_LEAF", "1")))
CHUNK = 512

DEV_LEVELS = list(range(DEPTH - 1, L_STOP - 1, -1))  # 16 .. L_STOP
LCOLS = {d: (2**d) // NCORES for d in DEV_LEVELS}  # per-core cols per level
XCOLS = sum(LCOLS.values())
XOFF = {}
_off = 0
for _d in DEV_LEVELS:
    XOFF[_d] = _off
    _off += LCOLS[_d]
TOPC = LCOLS[L_STOP]

F32 = mybir.dt.float32
F32R = mybir.dt.float32r

W_NAMES = ["Wi", "Wo", "Wu", "Wf", "Ui", "Uo", "Uu", "Uf"]
WOFF = {n: i * H for i, n in enumerate(W_NAMES)}


def _build_nc():
    nc = bacc.Bacc("TRN2", target_bir_lowering=False, debug=False)
    xT = nc.dram_tensor("xT", [H, XCOLS], F32R, kind="ExternalInput").ap()
    wT = nc.dram_tensor("wT", [H, 8 * H], F32R, kind="ExternalInput").ap()
    bias = nc.dram_tensor("bias", [H, 8], F32, kind="ExternalInput").ap()
    hc = nc.dram_tensor("hc", [H, 2 * TOPC], F32, kind="ExternalOutput").ap()

    Sig = mybir.ActivationFunctionType.Sigmoid
    Tanh = mybir.ActivationFunctionType.Tanh

    with tile.TileContext(nc) as tc:
        with (
            tc.tile_pool(name="const", bufs=1) as constp,
            tc.tile_pool(name="hcbuf", bufs=1) as hcp,
            tc.tile_pool(name="xin", bufs=4) as xinp,
            tc.tile_pool(name="gates", bufs=2) as gp,
            tc.tile_pool(name="ps2", bufs=2, space="PSUM") as ps2,
            tc.tile_pool(name="ps1", bufs=1, space="PSUM") as ps1,
        ):
            # weights/bias go through the gpsimd (SWDGE) queue so the x-chunk
            # stream on the sync queue starts immediately
            w_sb = constp.tile([H, 8 * H], F32R, tag="w")
            nc.sync.dma_start(out=w_sb[:, : 3 * H], in_=wT[:, : 3 * H])
            nc.gpsimd.dma_start(out=w_sb[:, 3 * H :], in_=wT[:, 3 * H :])
            b_sb = constp.tile([H, 8], F32, tag="b")
            nc.gpsimd.dma_start(out=b_sb, in_=bias)
            # warm the sigmoid/tanh ACT table at t=0 so the ~1.3us table load
            # is off the critical path of the first real activation
            warm = constp.tile([H, 1], F32, tag="warm")
            nc.vector.memset(warm, 0.0)
            nc.scalar.activation(
                warm, warm, mybir.ActivationFunctionType.Sigmoid
            )
            # bias cols: 0:bi_leaf 1:bo_leaf 2:bu_leaf 3:bi 4:bo 5:bu 6:bf

            # dedicated per-level h/c buffers: each written once, read once -
            # no write-after-read hazards serializing level boundaries
            hbuf = {
                d: hcp.tile([H, LCOLS[d]], F32R, tag=f"h{d}", name=f"h{d}")
                for d in DEV_LEVELS
            }
            cbuf = {
                d: hcp.tile([H, LCOLS[d]], F32, tag=f"c{d}", name=f"c{d}")
                for d in DEV_LEVELS
            }

            def wsl(name):
                return w_sb[:, WOFF[name] : WOFF[name] + H]

            mm = nc.tensor.matmul
            act = nc.scalar.activation
            tt = nc.vector

            # tanh(c)+h of a chunk are deferred by one chunk (software
            # pipelining) so the scalar engine never stalls on the vector
            # engine's c-chain for the chunk it just fed
            pending = []

            def flush_pending():
                while pending:
                    c_sl, o_ap, h_sl, Cp = pending.pop()
                    t_sb = gp.tile([H, CHUNK], F32, tag="t_sb", name="t_sb")
                    act(t_sb[:, :Cp], c_sl, Tanh)
                    tt.tensor_mul(h_sl, o_ap, t_sb[:, :Cp])

            for d in DEV_LEVELS:
                L = LCOLS[d]
                leaf = d == DEPTH - 1
                h_out, c_out = hbuf[d], cbuf[d]
                h_in, c_in = (None, None) if leaf else (hbuf[d + 1], cbuf[d + 1])
                # the child level's last deferred tanh/h must be emitted before
                # any matmul of this level reads it (deps follow emission order)
                flush_pending()
                step = 256 if L <= 1024 else CHUNK
                for a in range(0, L, step):
                    C = min(step, L - a)
                    x_t = xinp.tile([H, CHUNK], F32R, tag="x")
                    nc.sync.dma_start(
                        out=x_t[:, :C], in_=xT[:, XOFF[d] + a : XOFF[d] + a + C]
                    )
                    io_ps = ps2.tile([H, 2 * CHUNK], F32, tag="io")
                    u_ps = ps2.tile([H, CHUNK], F32, tag="u")
                    if leaf:
                        mm(io_ps[:, :C], wsl("Wi"), x_t[:, :C], start=True, stop=True)
                        mm(
                            io_ps[:, CHUNK : CHUNK + C],
                            wsl("Wo"),
                            x_t[:, :C],
                            start=True,
                            stop=True,
                        )
                        mm(u_ps[:, :C], wsl("Wu"), x_t[:, :C], start=True, stop=True)
                    else:
                        he = h_in[:, a : a + C]
                        ho = h_in[:, L + a : L + a + C]
                        isl = io_ps[:, :C]
                        osl = io_ps[:, CHUNK : CHUNK + C]
                        mm(isl, wsl("Wi"), x_t[:, :C], start=True, stop=False)
                        mm(isl, wsl("Ui"), he, start=False, stop=False)
                        mm(isl, wsl("Ui"), ho, start=False, stop=True)
                        mm(osl, wsl("Wo"), x_t[:, :C], start=True, stop=False)
                        mm(osl, wsl("Uo"), he, start=False, stop=False)
                        mm(osl, wsl("Uo"), ho, start=False, stop=True)
                        mm(u_ps[:, :C], wsl("Wu"), x_t[:, :C], start=True, stop=False)
                        mm(u_ps[:, :C], wsl("Uu"), he, start=False, stop=False)
                        mm(u_ps[:, :C], wsl("Uu"), ho, start=False, stop=True)
                        f_ps = ps1.tile([H, 2 * CHUNK], F32, tag="f")
                        f0 = f_ps[:, :C]
                        f1 = f_ps[:, CHUNK : CHUNK + C]
                        mm(f0, wsl("Wf"), x_t[:, :C], start=True, stop=False)
                        mm(f1, wsl("Wf"), x_t[:, :C], start=True, stop=False)
                        mm(f0, wsl("Uf"), he, start=False, stop=True)
                        mm(f1, wsl("Uf"), ho, start=False, stop=True)

                    io_sb = gp.tile([H, 2 * CHUNK], F32, tag="io_sb")
                    u_sb = gp.tile([H, CHUNK], F32, tag="u_sb")
                    bcol = 0 if leaf else 3
                    act(io_sb[:, :C], io_ps[:, :C], Sig, bias=b_sb[:, bcol : bcol + 1])
                    act(
                        io_sb[:, CHUNK : CHUNK + C],
                        io_ps[:, CHUNK : CHUNK + C],
                        Sig,
                        bias=b_sb[:, bcol + 1 : bcol + 2],
                    )
                    act(
                        u_sb[:, :C],
                        u_ps[:, :C],
                        Tanh,
                        bias=b_sb[:, bcol + 2 : bcol + 3],
                    )
                    c_sl = c_out[:, a : a + C]
                    if leaf:
                        if GPSIMD_LEAF:
                            nc.gpsimd.tensor_mul(c_sl, io_sb[:, :C], u_sb[:, :C])
                        else:
                            tt.tensor_mul(c_sl, io_sb[:, :C], u_sb[:, :C])
                        flush_pending()
                    else:
                        f_sb = gp.tile([H, 2 * CHUNK], F32, tag="f_sb")
                        if C == CHUNK:
                            act(f_sb, f_ps, Sig, bias=b_sb[:, 6:7])
                        else:
                            # both halves in one op via a [128, 2, C] pattern
                            act(
                                f_sb.rearrange("p (two c) -> p two c", two=2)[
                                    :, :, :C
                                ],
                                f_ps.rearrange("p (two c) -> p two c", two=2)[
                                    :, :, :C
                                ],
                                Sig,
                                bias=b_sb[:, 6:7],
                            )
                        q = gp.tile([H, CHUNK], F32, tag="q")
                        pr = gp.tile([H, 2 * CHUNK], F32, tag="pr")
                        s1 = gp.tile([H, CHUNK], F32, tag="s1")
                        nc.gpsimd.tensor_mul(q[:, :C], io_sb[:, :C], u_sb[:, :C])
                        # f0*c_even and f1*c_odd in one op via [128, 2, C] APs
                        tt.tensor_mul(
                            pr.rearrange("p (two c) -> p two c", two=2)[:, :, :C],
                            f_sb.rearrange("p (two c) -> p two c", two=2)[:, :, :C],
                            c_in.rearrange("p (two l) -> p two l", two=2)[
                                :, :, a : a + C
                            ],
                        )
                        tt.tensor_add(s1[:, :C], q[:, :C], pr[:, :C])
                        tt.tensor_add(c_sl, s1[:, :C], pr[:, CHUNK : CHUNK + C])
                        flush_pending()
                    pending.append(
                        (c_sl, io_sb[:, CHUNK : CHUNK + C], h_out[:, a : a + C], C)
                    )

            flush_pending()

            nc.sync.dma_start(out=hc[:, :TOPC], in_=hbuf[L_STOP].bitcast(F32))
            nc.sync.dma_start(out=hc[:, TOPC : 2 * TOPC], in_=cbuf[L_STOP])
    nc.finalize()
    return nc


_NC = None


def _get_nc():
    global _NC
    if _NC is None:
        _NC = _build_nc()
    return _NC


def _stored_cols(m):
    """Column order (node ids) of core m's xT buffer: levels 16..L_STOP,
    each level in even/odd-split order derived from the level above."""
    ids = np.arange(2**L_STOP - 1 + TOPC * m, 2**L_STOP - 1 + TOPC * (m + 1))
    per_level = {L_STOP: ids}
    for d in range(L_STOP, DEPTH - 1):
        ids = np.concatenate([2 * ids + 1, 2 * ids + 2])
        per_level[d + 1] = ids
    return np.concatenate([per_level[d] for d in DEV_LEVELS]), per_level


def _sigmoid(z):
    return 1.0 / (1.0 + np.exp(-z))


def kernel(**inputs):
    x = np.ascontiguousarray(np.asarray(inputs["x"], dtype=np.float32))
    wstack = np.ascontiguousarray(
        np.concatenate([np.asarray(inputs[n], np.float32).T for n in W_NAMES], axis=1)
    )
    b = {k: np.asarray(inputs[k], np.float64) for k in inputs if k.startswith("b")}
    bias = np.zeros((H, 8), np.float32)
    bias[:, 0] = b["bWi"]
    bias[:, 1] = b["bWo"]
    bias[:, 2] = b["bWu"]
    bias[:, 3] = b["bWi"] + b["bUi"]
    bias[:, 4] = b["bWo"] + b["bUo"]
    bias[:, 5] = b["bWu"] + b["bUu"]
    bias[:, 6] = b["bWf"] + b["bUf"]

    in_maps = []
    for m in range(NCORES):
        cols, _ = _stored_cols(m)
        in_maps.append(
            {
                "xT": np.ascontiguousarray(x[cols].T),
                "wT": wstack,
                "bias": bias,
            }
        )

    nc = _get_nc()
    trace = bool(int(os.environ.get("KERNEL_TRACE", "0")))
    try:
        res = run_bass_kernel_spmd(
            nc, in_maps, core_ids=list(range(NCORES)), trace=trace
        )
    except ModuleNotFoundError:
        res = run_bass_kernel_spmd(nc, in_maps, core_ids=list(range(NCORES)))
    if trace and res.exec_time_ns is not None:
        print(f"HW exec time: {res.exec_time_ns} ns")

    # stored level-L_STOP columns of core m are the natural-order nodes
    # 2**L_STOP - 1 + 32*m ...  (that's how _stored_cols seeds them)
    h_next = np.concatenate(
        [res.results[m]["hc"][:, :TOPC] for m in range(NCORES)], axis=1
    ).T.astype(np.float64)
    c_next = np.concatenate(
        [res.results[m]["hc"][:, TOPC : 2 * TOPC] for m in range(NCORES)], axis=1
    ).T.astype(np.float64)

    # finish levels L_STOP-1 .. 0 on the host (float64)
    xd = x.astype(np.float64)
    W = {n: np.asarray(inputs[n], np.float64) for n in W_NAMES}
    for d in range(L_STOP - 1, -1, -1):
        s = 2**d - 1
        cnt = 2**d
        xs = xd[s : s + cnt]
        li = xs @ W["Wi"].T + b["bWi"]
        lf = xs @ W["Wf"].T + b["bWf"]
        lo = xs @ W["Wo"].T + b["bWo"]
        lu = xs @ W["Wu"].T + b["bWu"]
        ch_h = h_next.reshape(cnt, 2, H)
        ch_c = c_next.reshape(cnt, 2, H)
        hs = ch_h[:, 0, :] + ch_h[:, 1, :]
        i = _sigmoid(li + hs @ W["Ui"].T + b["bUi"])
        o = _sigmoid(lo + hs @ W["Uo"].T + b["bUo"])
        u = np.tanh(lu + hs @ W["Uu"].T + b["bUu"])
        f0 = _sigmoid(lf + ch_h[:, 0, :] @ W["Uf"].T + b["bUf"])
        f1 = _sigmoid(lf + ch_h[:, 1, :] @ W["Uf"].T + b["bUf"])
        c = i * u + f0 * ch_c[:, 0, :] + f1 * ch_c[:, 1, :]
        h = o * np.tanh(c)
        h_next, c_next = h, c

    out = h_next[0] @ np.asarray(inputs["Wp"], np.float64).T + np.asarray(
        inputs["bWp"], np.float64
    )
    return out.astype(np.float32)
